# revision 27
# baseline (speedup 1.0000x reference)
"""Additive (Bahdanau) attention scoring kernel for Trainium2, 8-core SPMD.

Reference computation (B=16, S=4096, D=1024, all fp32):
    q      = target @ Wq.T                    # [B, D]
    k      = memory @ Wk.T                    # [B, S, D]
    scores = tanh(q[:, None, :] + k) @ v      # [B, S]
    out    = softmax(scores - 1e9 * mask, axis=-1)

Sharding: batch across the 8 cores (2 batches per core), weights replicated.

Host-side prep is layout/quantize only (no arithmetic): memory is
transposed to [D, S] per batch, compacted to the unmasked positions
(masked positions contribute exactly 0 to the softmax since exp(-1e9)
underflows to 0 in fp32, so dropping them is algebraically exact),
cast to bf16, and pre-tiled so each DMA is one contiguous block.

Device layout ("s on partitions"): compact positions are processed in
s-tiles of 128. For each s-tile the PE computes k^T as
  k_ps[s=128, e=1024] += memtile[d=128, s=128].T @ WkT[d=128, e]
accumulated over the 8 d-chunks (16 matmuls of N=512, mem stationary,
Wk moving, all bf16). Everything else runs off the PE:
  - DVE adds q (materialized once per batch as a [128, 1024] tile via a
    K=1 ones-matmul against the on-device q = Wq-matmul result),
  - ACT applies tanh (fp32 PSUM -> bf16 SBUF),
  - DVE scalar_tensor_tensor multiplies by the broadcast v and its
    accum_out (per-partition free-axis sum) yields the 128 scores.
Finale per batch: add a 0/-1e9 pad-penalty tile (pads -> exp == 0),
ACT Exp with accum_out -> row sums, one 128x128 ones-matmul
reduces+broadcasts the total, DVE reciprocal + scale, DMA out the
compact [128, ST] probabilities. The host scatters them to full S
(pure indexing; masked positions are exactly 0).
"""

import math
import os
from contextlib import ExitStack

import ml_dtypes
import numpy as np

import concourse.tile as tile
from concourse import bacc, mybir
import concourse.bass as bass  # noqa: F401  (kept for parity with harness imports)

B, S, D = 16, 4096, 1024
N_CORES = 8
NB = B // N_CORES  # batches per core
P = 128
DC = D // P        # contraction chunks
EH = D // 512      # moving-operand halves (PSUM bank = 512 fp32)

F32 = mybir.dt.float32
BF16 = mybir.dt.bfloat16
AF = mybir.ActivationFunctionType
MUL = mybir.AluOpType.mult

BF16NP = ml_dtypes.bfloat16

_CACHE = {}


def _chunks(ST):
    """DMA chunks of up to 4 s-tiles (1 MiB of bf16 per full chunk). The
    first chunk is a single tile so the PE's first matmul dependency is a
    256 KiB transfer."""
    return [(0, 1)] + [(i, min(4, ST - i)) for i in range(1, ST, 4)]


def _build_program(ST, stage):
    s_pad = ST * P
    chunks = _chunks(ST)

    nc = bacc.Bacc("TRN2", target_bir_lowering=False, debug=False)

    memC = nc.dram_tensor("memC", [NB, P, DC * s_pad], BF16, kind="ExternalInput").ap()
    wkT = nc.dram_tensor("wkT", [P, DC * D], BF16, kind="ExternalInput").ap()
    wqT = nc.dram_tensor("wqT", [P, DC * D], BF16, kind="ExternalInput").ap()
    tgtT = nc.dram_tensor("tgtT", [P, DC * NB], BF16, kind="ExternalInput").ap()
    vbc = nc.dram_tensor("vbc", [P, D], BF16, kind="ExternalInput").ap()
    pen = nc.dram_tensor("pen", [NB, P, ST], F32, kind="ExternalInput").ap()
    sel = nc.dram_tensor("sel", [NB, NB * P], F32, kind="ExternalInput").ap()
    out = nc.dram_tensor("out", [NB, P, ST], F32, kind="ExternalOutput").ap()

    with tile.TileContext(nc) as tc, ExitStack() as ctx:
        consts = ctx.enter_context(tc.tile_pool(name="consts", bufs=1))
        mem_pool = ctx.enter_context(tc.tile_pool(name="mem", bufs=3))
        ti_pool = ctx.enter_context(tc.tile_pool(name="ti", bufs=3))
        tt_pool = ctx.enter_context(tc.tile_pool(name="tt", bufs=3))
        ttv_pool = ctx.enter_context(tc.tile_pool(name="ttv", bufs=2))
        fin_pool = ctx.enter_context(tc.tile_pool(name="fin", bufs=2))
        kps_pool = ctx.enter_context(tc.tile_pool(name="kps", bufs=3, space="PSUM"))
        qps_pool = ctx.enter_context(tc.tile_pool(name="qps", bufs=1, space="PSUM"))

        # --- constants / weights (issue order = DMA priority order: the
        # first memC chunk + Wk gate the PE's first k-matmuls; Wq is only
        # needed ~14us in; vbc/pen even later) ---
        w0 = chunks[0][1] * P
        mem0_sb = mem_pool.tile([P, DC * 4 * P], BF16, tag="mem", name="mem_sb")
        nc.sync.dma_start(mem0_sb[:, :DC * w0], memC[0, :, 0:DC * w0])
        tgt_sb = consts.tile([P, DC * NB], BF16)
        nc.sync.dma_start(tgt_sb[:], tgtT[:, :])
        sel_sb = consts.tile([NB, NB * P], F32)
        nc.sync.dma_start(sel_sb[:], sel[:, :])
        # weights go out on the second HWDGE ring (ACT) so their transfers
        # and fixed completion costs overlap the memC stream on sync's ring
        wk_sb = consts.tile([P, DC * D], BF16)
        for dc in range(DC):
            nc.scalar.dma_start(
                wk_sb[:, dc * D:(dc + 1) * D], wkT[:, dc * D:(dc + 1) * D]
            )
        wq_sb = consts.tile([P, DC * D], BF16)
        for h in range(2):
            nc.scalar.dma_start(
                wq_sb[:, h * 4 * D:(h + 1) * 4 * D], wqT[:, h * 4 * D:(h + 1) * 4 * D]
            )
        vbc_sb = consts.tile([P, D], BF16)
        nc.scalar.dma_start(vbc_sb[:], vbc[:, :])
        pen_sb = consts.tile([P, NB * ST], F32)
        for b in range(NB):
            nc.scalar.dma_start(pen_sb[:, b * ST:(b + 1) * ST], pen[b])

        ones128 = consts.tile([P, P], F32)
        nc.vector.memset(ones128[:], 1.0)

        q_sb = consts.tile([NB, D], F32)
        qt_sb = consts.tile([P, NB * D], F32)
        score_sbs = [consts.tile([P, ST], F32, tag=f"score{b}", name=f"score{b}")
                     for b in range(NB)]
        rs_sbs = [consts.tile([P, 1], F32, tag=f"rs{b}", name=f"rs{b}") for b in range(NB)]
        rc_sbs = [consts.tile([P, 1], F32, tag=f"rc{b}", name=f"rc{b}") for b in range(NB)]

        def emit_qsetup():
            # q = target @ Wq.T on the PE, fp32 accumulate -> [NB, D]
            q_ps = qps_pool.tile([P, D], F32, tag="qps", name="q_ps")
            for eh in range(EH):
                for dc in range(DC):
                    nc.tensor.matmul(
                        q_ps[0:NB, eh * 512:(eh + 1) * 512],
                        tgt_sb[:, dc * NB:(dc + 1) * NB],
                        wq_sb[:, dc * D + eh * 512: dc * D + (eh + 1) * 512],
                        start=(dc == 0),
                        stop=(dc == DC - 1),
                    )
            nc.vector.tensor_copy(q_sb[:], q_ps[0:NB, :])
            # q_tile[b]: [128, 1024] with every row equal to q[b], via a K=2
            # selector matmul (row b of q_sb picked by the 0/1 selector, so
            # the moving operand stays at base partition 0).
            for b in range(NB):
                qt_ps = qps_pool.tile([P, D], F32, tag="qps", name="qt_ps")
                for eh in range(EH):
                    nc.tensor.matmul(
                        qt_ps[:, eh * 512:(eh + 1) * 512],
                        sel_sb[:, b * P:(b + 1) * P],
                        q_sb[0:NB, eh * 512:(eh + 1) * 512],
                        start=True,
                        stop=True,
                    )
                nc.vector.tensor_copy(qt_sb[:, b * D:(b + 1) * D], qt_ps[:])

        def emit_tile_mm(mem_sb, w, t):
            k_ps = kps_pool.tile([P, D], F32, tag="kps", name="k_ps")
            for dc in range(DC):
                for eh in range(EH):
                    nc.tensor.matmul(
                        k_ps[:, eh * 512:(eh + 1) * 512],
                        mem_sb[:, dc * w + t * P: dc * w + (t + 1) * P],
                        wk_sb[:, dc * D + eh * 512: dc * D + (eh + 1) * 512],
                        start=(dc == 0),
                        stop=(dc == DC - 1),
                    )
            return k_ps

        def emit_tile_post(b, k_ps, j):
            ti = ti_pool.tile([P, D], F32, tag="ti", name="ti")
            nc.vector.tensor_add(ti[:], k_ps[:], qt_sb[:, b * D:(b + 1) * D])
            tt = tt_pool.tile([P, D], BF16, tag="tt", name="tt")
            nc.scalar.activation(tt[:], ti[:], AF.Tanh)
            ttv = ttv_pool.tile([P, D], BF16, tag="ttv", name="ttv")
            nc.vector.scalar_tensor_tensor(
                ttv[:], tt[:], 1.0, vbc_sb[:],
                op0=MUL, op1=MUL,
                accum_out=score_sbs[b][:, j:j + 1],
            )

        # --- main loop. The first 3 tiles' k-matmuls (= kps pool depth) are
        # emitted before the q-setup so the PE can start as soon as Wk +
        # memC chunk 0 land; their DVE/ACT post-processing (which needs
        # q_tile) follows the q-setup. ---
        exs = []
        for b in range(NB):
            for ci, (coff, cnt) in enumerate(chunks):
                w = cnt * P
                if b == 0 and ci == 0:
                    mem_sb = mem0_sb
                else:
                    mem_sb = mem_pool.tile([P, DC * 4 * P], BF16, tag="mem", name="mem_sb")
                    nc.sync.dma_start(
                        mem_sb[:, :DC * w],
                        memC[b, :, DC * coff * P: DC * coff * P + DC * w],
                    )
                if b == 0 and ci == 1:
                    # 3 tiles of k-matmuls (= kps depth) run before the
                    # q-setup; their post-processing (which needs q_tile)
                    # follows it
                    leads = [emit_tile_mm(mem_sb, w, t) for t in range(2)]
                    emit_qsetup()
                    emit_tile_post(b, lead0, 0)
                    for t in range(2):
                        emit_tile_post(b, leads[t], coff + t)
                    for t in range(2, cnt):
                        emit_tile_post(b, emit_tile_mm(mem_sb, w, t), coff + t)
                    continue
                if b == 0 and ci == 0:
                    lead0 = emit_tile_mm(mem_sb, w, 0)
                    continue
                for t in range(cnt):
                    emit_tile_post(b, emit_tile_mm(mem_sb, w, t), coff + t)
            # per-batch finale front half (DVE/ACT only, so it interleaves
            # into the other batch's compute; the PE part comes at the end)
            sm = fin_pool.tile([P, ST], F32, tag=f"sm{b}", name=f"sm{b}")
            nc.vector.tensor_add(sm[:], score_sbs[b][:], pen_sb[:, b * ST:(b + 1) * ST])
            if stage >= 2:
                ex = fin_pool.tile([P, ST], F32, tag=f"ex{b}", name=f"ex{b}")
                nc.scalar.activation(ex[:], sm[:], AF.Exp, accum_out=rs_sbs[b][:])
                exs.append(ex)
            else:
                exs.append(sm)

        # --- finale back half (the only PE/DVE work after the last k-matmul)
        for b in range(NB):
            if stage < 2:
                nc.sync.dma_start(out[b], exs[b][:])
                continue
            tot_ps = qps_pool.tile([P, D], F32, tag="qps", name="tot_ps")
            nc.tensor.matmul(tot_ps[:, 0:1], ones128[:], rs_sbs[b][:], start=True, stop=True)
            nc.vector.reciprocal(rc_sbs[b][:], tot_ps[:, 0:1])
            ot = fin_pool.tile([P, ST], F32, tag=f"ot{b}", name=f"ot{b}")
            nc.vector.tensor_scalar_mul(ot[:], exs[b][:], rc_sbs[b][:])
            nc.sync.dma_start(out[b], ot[:])

    nc.compile()
    return nc


def get_program(ST=None, stage=None):
    if stage is None:
        stage = int(os.environ.get("KERNEL_STAGE", "2"))
    assert ST is not None
    key = (ST, stage)
    if key not in _CACHE:
        _CACHE[key] = _build_program(ST, stage)
    return _CACHE[key]


def prepare_in_maps(memory, target, memory_mask, Wq, Wk, v):
    memory = np.asarray(memory, dtype=np.float32)
    target = np.asarray(target, dtype=np.float32)
    Wq = np.asarray(Wq, dtype=np.float32)
    Wk = np.asarray(Wk, dtype=np.float32)
    v = np.asarray(v, dtype=np.float32)
    mask = np.asarray(memory_mask)

    keep = ~mask                                   # [B, S]
    counts = keep.sum(1).astype(np.int64)
    max_kept = int(counts.max())
    ST = math.ceil(max_kept / P)
    s_pad = ST * P
    chunks = _chunks(ST)

    kept_idx = []
    memC = np.empty((B, P, DC * s_pad), dtype=BF16NP)
    for b in range(B):
        idx = np.flatnonzero(keep[b])
        kept_idx.append(idx)
        pad = np.empty(s_pad, dtype=np.int64)
        pad[:len(idx)] = idx
        pad[len(idx):] = idx[0]
        # [D, s_pad] -> [P, DC, s_pad] (partition = d % 128) -> chunk-major
        A = memory[b][pad].T.astype(BF16NP).reshape(DC, P, s_pad).transpose(1, 0, 2)
        blocks = [
            np.ascontiguousarray(A[:, :, off * P:(off + cnt) * P]).reshape(P, DC * cnt * P)
            for (off, cnt) in chunks
        ]
        memC[b] = np.concatenate(blocks, axis=1)

    def chunked_T(W):  # [D, D] -> [P, DC*D] with partition = d % 128
        return np.ascontiguousarray(
            W.T.astype(BF16NP).reshape(DC, P, D).transpose(1, 0, 2).reshape(P, DC * D)
        )

    wkT = chunked_T(Wk)
    wqT = chunked_T(Wq)
    tgtT_full = target.T.astype(BF16NP).reshape(DC, P, B).transpose(1, 0, 2)  # [P, DC, B]
    vbc_arr = np.ascontiguousarray(np.broadcast_to(v.astype(BF16NP), (P, D)))

    # pad penalty: position j*128 + p is real iff < counts[b]
    pos = (np.arange(ST)[None, :] * P + np.arange(P)[:, None])  # [P, ST]
    pen = np.where(pos[None, :, :] < counts[:, None, None], 0.0, -1e9).astype(np.float32)

    # batch-row selector for the q_tile matmul: sel[i, b*P + s] = (i == b)
    sel = np.zeros((NB, NB * P), dtype=np.float32)
    for b in range(NB):
        sel[b, b * P:(b + 1) * P] = 1.0

    in_maps = [
        {
            "memC": np.ascontiguousarray(memC[c * NB:(c + 1) * NB]),
            "wkT": wkT,
            "wqT": wqT,
            "tgtT": np.ascontiguousarray(
                tgtT_full[:, :, c * NB:(c + 1) * NB].reshape(P, DC * NB)
            ),
            "vbc": vbc_arr,
            "pen": np.ascontiguousarray(pen[c * NB:(c + 1) * NB]),
            "sel": sel,
        }
        for c in range(N_CORES)
    ]
    meta = {"ST": ST, "counts": counts, "kept_idx": kept_idx}
    return in_maps, meta


def gather_output(results, meta):
    ST = meta["ST"]
    out = np.zeros((B, S), dtype=np.float32)
    for c in range(N_CORES):
        arr = np.asarray(results[c]["out"], dtype=np.float32)  # [NB, P, ST]
        for i in range(NB):
            b = c * NB + i
            compact = arr[i].T.reshape(ST * P)  # position j*128+p at [p, j]
            idx = meta["kept_idx"][b]
            out[b, idx] = compact[:len(idx)]
    return out


def kernel(memory, target, memory_mask, Wq, Wk, v):
    from concourse.bass_utils import run_bass_kernel_spmd

    in_maps, meta = prepare_in_maps(memory, target, memory_mask, Wq, Wk, v)
    nc = get_program(ST=meta["ST"])
    res = run_bass_kernel_spmd(nc, in_maps, list(range(N_CORES)))
    return gather_output(res.results, meta)
